# revision 1
# baseline (speedup 1.0000x reference)
"""PSNR-conv kernel for TRN2 (8 NeuronCores, SPMD).

Problem: per 16x16 window of a 4x2048x2048 image, alpha-blend with a 16x16
RGBA kernel and compute PSNR vs the kernel, averaged over channels.
Mathematically per channel c:
    mse_c = sum_ij w'_ij * (x_c[i,j] - k_c[i,j])^2,   w' = ((1-a)^2)/256
          = conv(x_c^2, w') - conv(x_c, 2 k_c w') + sum(k_c^2 w')
    out   = A - B * sum_c ln(mse_c),  A = 20 log10(255), B = 10/(4 ln 10)

Implementation: each depthwise 16x16 VALID conv is computed as 16 banded-
Toeplitz matmuls on the PE array (contraction over 128 input rows, lhsT =
Toeplitz of one kernel column, rhs = image tile shifted by dj in the free
dim), accumulating both conv planes (x^2 and x) into one PSUM tile so the
PSUM holds mse_c - skk_c directly. ScalarE Ln(+skk bias) + VectorE combine
produce the output tile.

Sharding: 2x4 grid (2 row strips x 4 col strips). Every core runs an
identical instruction stream (SPMD, same NEFF): 9 row blocks of 113 output
rows x 509 output cols. Strips overlap by a few rows/cols (recomputed) so
all cores are uniform. Inputs are fp32r (e8m11) for 1 cycle/row matmuls.
"""

import sys

if "/opt/trn_rl_repo" not in sys.path:
    sys.path.insert(0, "/opt/trn_rl_repo")

import numpy as np

PIXEL_MAX = 255.0
C, Hk, Wk = 4, 16, 16
H = W = 2048
HO = WO = H - Hk + 1          # 2033
MB = 113                      # output rows per block (128 - 15)
KP = 128                      # contraction size (input rows per block)
NRB = 9                       # row blocks per core; 9*113 = 1017 rows
OUT_ROWS = NRB * MB           # 1017
NCOL = 510                    # output cols per core (fp32r mm: must be even)
IN_COLS = NCOL + Hk - 1       # 525
IN_ROWS = OUT_ROWS + Hk - 1   # 1032
ROW_STARTS = [0, HO - OUT_ROWS]                    # [0, 1016]
COL_STARTS = [0, 507, 1015, WO - NCOL]             # [0, 507, 1015, 1523]
N_CORES = 8

A_CONST = 20.0 * np.log10(PIXEL_MAX)
B_CONST = 10.0 / (4.0 * np.log(10.0))


def _toeplitz(col):
    """[128, 113] banded Toeplitz T[k, m] = col[k - m] for 0 <= k-m < 16."""
    t = np.zeros((KP, MB), np.float32)
    for d in range(Hk):
        idx = np.arange(MB)
        t[idx + d, idx] = col[d]
    return t


def _build_nc(skk):
    import concourse.bacc as bacc
    import concourse.mybir as mybir
    from concourse.tile import TileContext

    f32 = mybir.dt.float32
    f32r = mybir.dt.float32r
    Ln = mybir.ActivationFunctionType.Ln
    mult = mybir.AluOpType.mult
    add = mybir.AluOpType.add

    nc = bacc.Bacc("TRN2", target_bir_lowering=False, debug=False)

    # Register const APs for the per-channel skk activation biases (the
    # ACT bias-as-float path looks immediates up in const_aps).
    for v in skk:
        v = float(v)
        t = nc.alloc_sbuf_tensor(f"const-float32-{v}", [128, 1], f32)
        nc.gpsimd.memset(t.ap(), v)
        nc.const_aps.aps[(f32, v)] = t.ap()
    nc.all_engine_barrier()

    xs = nc.dram_tensor("xs", [C, IN_ROWS, IN_COLS], f32, kind="ExternalInput")
    tw = nc.dram_tensor("tw", [Hk, KP, MB], f32, kind="ExternalInput")
    tkw = nc.dram_tensor("tkw", [C, Hk, KP, MB], f32, kind="ExternalInput")
    out = nc.dram_tensor("out", [OUT_ROWS, NCOL], f32, kind="ExternalOutput")

    with TileContext(nc) as tc:
        with (
            tc.tile_pool(name="wpool", bufs=1) as wpool,
            tc.tile_pool(name="xpool", bufs=2) as xpool,
            tc.tile_pool(name="lnpool", bufs=2) as lnpool,
            tc.tile_pool(name="opool", bufs=2) as opool,
            tc.tile_pool(name="pspool", bufs=2, space="PSUM") as pspool,
        ):
            # one-time: weights -> SBUF, round to fp32r
            tw_st = wpool.tile([KP, Hk * MB], f32)
            tkw_st = wpool.tile([KP, C * Hk * MB], f32)
            nc.sync.dma_start(
                tw_st[:].rearrange("k (d m) -> k d m", d=Hk),
                tw[:].rearrange("d k m -> k d m"),
            )
            nc.sync.dma_start(
                tkw_st[:].rearrange("k (c d m) -> k c d m", c=C, d=Hk),
                tkw[:].rearrange("c d k m -> k c d m"),
            )
            twr = wpool.tile([KP, Hk * MB], f32r)
            tkwr = wpool.tile([KP, C * Hk * MB], f32r)
            nc.vector.tensor_copy(twr[:], tw_st[:])
            nc.vector.tensor_copy(tkwr[:], tkw_st[:])

            for rb in range(NRB):
                row0 = MB * rb
                xt = xpool.tile([KP, C * IN_COLS], f32, tag="xt")
                nc.sync.dma_start(
                    xt[:].rearrange("r (c w) -> r c w", c=C),
                    xs[:, row0:row0 + KP, :].rearrange("c r w -> r c w"),
                )
                xxr = xpool.tile([KP, C * IN_COLS], f32r, tag="xxr")
                xr = xpool.tile([KP, C * IN_COLS], f32r, tag="xr")
                nc.vector.tensor_mul(xxr[:], xt[:], xt[:])
                nc.vector.tensor_copy(xr[:], xt[:])

                lns = []
                for c in range(C):
                    ps = pspool.tile([MB, NCOL], f32, tag=f"ps{c}", name=f"ps{c}")
                    for dj in range(Hk):
                        nc.tensor.matmul(
                            ps[:],
                            twr[:, dj * MB:(dj + 1) * MB],
                            xxr[:, c * IN_COLS + dj: c * IN_COLS + dj + NCOL],
                            start=(dj == 0), stop=False,
                        )
                    for dj in range(Hk):
                        nc.tensor.matmul(
                            ps[:],
                            tkwr[:, (c * Hk + dj) * MB:(c * Hk + dj + 1) * MB],
                            xr[:, c * IN_COLS + dj: c * IN_COLS + dj + NCOL],
                            start=False, stop=(dj == Hk - 1),
                        )
                    lnc = lnpool.tile([MB, NCOL], f32, tag=f"ln{c}", name=f"ln{c}")
                    nc.scalar.activation(
                        lnc[:], ps[:], Ln, bias=float(skk[c]), scale=1.0
                    )
                    lns.append(lnc)

                s01 = opool.tile([MB, NCOL], f32, tag="s01")
                s23 = opool.tile([MB, NCOL], f32, tag="s23")
                nc.vector.tensor_add(s01[:], lns[0][:], lns[1][:])
                nc.vector.tensor_add(s23[:], lns[2][:], lns[3][:])
                t = opool.tile([MB, NCOL], f32, tag="t")
                nc.vector.tensor_add(t[:], s01[:], s23[:])
                ob = opool.tile([MB, NCOL], f32, tag="ob")
                nc.vector.tensor_scalar(
                    ob[:], t[:], -B_CONST, A_CONST, mult, add
                )
                nc.sync.dma_start(out[row0:row0 + MB, :], ob[:])

    nc.compile()
    return nc


def kernel(x, kernel):
    from concourse.bass_utils import run_bass_kernel_spmd

    x = np.asarray(x)
    kernel = np.asarray(kernel)
    k = kernel[0].astype(np.float64)                    # (4, 16, 16)
    alpha = k[3] / PIXEL_MAX
    wp = ((1.0 - alpha) ** 2) / (Hk * Wk)               # w' = (1-a)^2 / 256

    tw_np = np.zeros((Hk, KP, MB), np.float32)
    tkw_np = np.zeros((C, Hk, KP, MB), np.float32)
    for dj in range(Hk):
        tw_np[dj] = _toeplitz(wp[:, dj].astype(np.float32))
        for c in range(C):
            tkw_np[c, dj] = _toeplitz((-2.0 * k[c, :, dj] * wp[:, dj]).astype(np.float32))
    skk = (k * k * wp).sum(axis=(-2, -1))               # (4,)

    nc = _build_nc(skk)

    x0 = np.ascontiguousarray(x[0], np.float32)         # (4, 2048, 2048)
    in_maps = []
    for r in range(2):
        for cc in range(4):
            r0, c0 = ROW_STARTS[r], COL_STARTS[cc]
            in_maps.append({
                "xs": np.ascontiguousarray(
                    x0[:, r0:r0 + IN_ROWS, c0:c0 + IN_COLS]
                ),
                "tw": tw_np,
                "tkw": tkw_np,
            })

    res = run_bass_kernel_spmd(nc, in_maps, core_ids=list(range(N_CORES)))

    full = np.empty((HO, WO), np.float32)
    for r in range(2):
        for cc in range(4):
            core = r * 4 + cc
            r0, c0 = ROW_STARTS[r], COL_STARTS[cc]
            full[r0:r0 + OUT_ROWS, c0:c0 + NCOL] = res.results[core]["out"]
    return full



# revision 2
# speedup vs baseline: 94.2453x; 94.2453x over previous
"""PSNR-conv kernel for TRN2 (8 NeuronCores, SPMD) — fp8 DoubleRow version.

Math (per channel c, per 16x16 window):
    mse_c*256 = sum w*x^2 - 2 sum k_c*w*x + sum k_c^2*w
Centered at y = x - 128 (halves fp8 quantization error):
    mse_c*256 = sum w*y^2 + sum d_c*y + const_c,
      d_c = 2w*(128 - k_c),  const_c = 16384*sum(w) - 256*sum(k_c w) + sum(k_c^2 w)
Both convs are fused into ONE fp8e4 DoubleRow matmul per kernel column dj:
contraction K=256 = [y^2-plane band | y-plane band], each a 16-tap Toeplitz
band over 128 input rows. PSUM accumulates (sum w y^2 + sum d y)/64 over the
16 dj. ScalarE Ln(+const/64 bias) and VectorE combine produce the output.

Scales: lhsT = [fp8(2w) | fp8(d/8)], rhs = [fp8(y^2/128) | fp8(y/8)] so both
k-tiles contribute at 1/64 scale. On-device prep: ACT Square(x/sqrt(128) -
sqrt(128)) -> y^2/128 (fp8), DVE x*(1/8) - 16 -> y/8 (fp8).

DoubleRow ISA restriction: the k-tile-pair stride (step_elem[2]) must be a
multiple of 16 elements on BOTH Ldweights and Matmult sides — hence weight
columns padded to MBP=128 and per-channel feature width padded to CIP=528.

Sharding: 2x4 grid of overlapping strips, SPMD identical instruction stream,
9 row blocks of 113 output rows x 510 output cols per core.
"""

import sys

if "/opt/trn_rl_repo" not in sys.path:
    sys.path.insert(0, "/opt/trn_rl_repo")

import numpy as np
import ml_dtypes

PIXEL_MAX = 255.0
C, Hk, Wk = 4, 16, 16
H = W = 2048
HO = WO = H - Hk + 1          # 2033
MB = 113                      # output rows per block (128 - 15)
MBP = 128                     # padded weight columns (stride mult of 16)
KP = 128                      # contraction rows per k-tile
NRB = 9                       # row blocks per core; 9*113 = 1017 rows
OUT_ROWS = NRB * MB           # 1017
NCOL = 510                    # output cols per core
IN_COLS = NCOL + Hk - 1       # 525
CIP = 528                     # padded feature width (stride mult of 16)
IN_ROWS = OUT_ROWS + Hk - 1   # 1032
ROW_STARTS = [0, HO - OUT_ROWS]                    # [0, 1016]
COL_STARTS = [0, 507, 1015, WO - NCOL]             # [0, 507, 1015, 1523]
N_CORES = 8

A_CONST = 20.0 * np.log10(PIXEL_MAX)
B_CONST = 10.0 / (4.0 * np.log(10.0))
# ln(mse) = Ln(psum + const/64) - ln 4  =>  fold 4*B*ln4 into the A constant
A_EFF = A_CONST + 4.0 * B_CONST * np.log(4.0)
SQ_SCALE = float(1.0 / np.sqrt(128.0))
SQ_BIAS = float(-np.sqrt(128.0))

F8 = ml_dtypes.float8_e4m3


def _build_nc(biases, reps=1):
    """biases: 4 floats, const_c/64 for the Ln activation.

    reps > 1 repeats the whole body inside one NEFF (idempotent — same
    output written each rep); used only for timing, where the marginal
    cost per extra rep isolates pure on-device body time."""
    import concourse.bacc as bacc
    import concourse.mybir as mybir
    from concourse.tile import TileContext

    f32 = mybir.dt.float32
    f8 = mybir.dt.float8e4
    Ln = mybir.ActivationFunctionType.Ln
    Square = mybir.ActivationFunctionType.Square
    mult = mybir.AluOpType.mult
    add = mybir.AluOpType.add
    DR = mybir.MatmulPerfMode.DoubleRow

    nc = bacc.Bacc("TRN2", target_bir_lowering=False, debug=False)

    # Register const APs for activation biases (ACT bias-as-float looks
    # immediates up in const_aps).
    for v in list(biases) + [SQ_BIAS]:
        v = float(v)
        t = nc.alloc_sbuf_tensor(f"const-float32-{v}", [128, 1], f32)
        nc.gpsimd.memset(t.ap(), v)
        nc.const_aps.aps[(f32, v)] = t.ap()
    nc.all_engine_barrier()

    xs = nc.dram_tensor("xs", [C, IN_ROWS, IN_COLS], f32, kind="ExternalInput")
    # DoubleRow Toeplitz weights, k-major so the DMA is contiguous per row:
    # [KP, C, Hk, 2, MBP] flattened to [KP, C*Hk*2*MBP]
    w8 = nc.dram_tensor("w8", [KP, C * Hk * 2 * MBP], f8, kind="ExternalInput")
    out = nc.dram_tensor("out", [OUT_ROWS, NCOL], f32, kind="ExternalOutput")

    with TileContext(nc) as tc:
        with (
            tc.tile_pool(name="wpool", bufs=1) as wpool,
            tc.tile_pool(name="xpool", bufs=2) as xpool,
            tc.tile_pool(name="fpool", bufs=2) as fpool,
            tc.tile_pool(name="lnpool", bufs=2) as lnpool,
            tc.tile_pool(name="opool", bufs=2) as opool,
            tc.tile_pool(name="pspool", bufs=2, space="PSUM") as pspool,
        ):
            wt = wpool.tile([KP, C * Hk * 2 * MBP], f8)
            nc.sync.dma_start(wt[:], w8[:])
            wv = wt[:].rearrange(
                "k (c d two m) -> k c d two m", c=C, d=Hk, two=2
            )

            for rb in range(NRB * reps):
                rb = rb % NRB
                row0 = MB * rb
                xt = xpool.tile([KP, C * IN_COLS], f32, tag="xt")
                nc.sync.dma_start(
                    xt[:].rearrange("r (c w) -> r c w", c=C),
                    xs[:, row0:row0 + KP, :].rearrange("c r w -> r c w"),
                )
                xv = xt[:].rearrange("k (c w) -> k c w", c=C)
                # fp8 feature tile [KP, C, 2, CIP]: per channel
                # [ y^2/128 | y/8 ], k-tile-pair stride CIP = 528
                ft = fpool.tile([KP, C * 2 * CIP], f8, tag="ft")
                fw = ft[:].rearrange("k (c two w) -> k c two w", c=C, two=2)
                nc.scalar.activation(
                    fw[:, :, 0, 0:IN_COLS], xv, Square,
                    bias=SQ_BIAS, scale=SQ_SCALE,
                )
                nc.vector.tensor_scalar(
                    fw[:, :, 1, 0:IN_COLS], xv, 0.125, -16.0, mult, add
                )

                lns = []
                for c in range(C):
                    ps = pspool.tile([MB, NCOL], f32, tag=f"ps{c}", name=f"ps{c}")
                    for dj in range(Hk):
                        nc.tensor.matmul(
                            ps[:],
                            wv[:, c, dj, :, 0:MB],
                            fw[:, c, :, dj:dj + NCOL],
                            start=(dj == 0), stop=(dj == Hk - 1),
                            perf_mode=DR,
                        )
                    lnc = lnpool.tile([MB, NCOL], f32, tag=f"ln{c}", name=f"ln{c}")
                    nc.scalar.activation(
                        lnc[:], ps[:], Ln, bias=float(biases[c]), scale=1.0
                    )
                    lns.append(lnc)

                s01 = opool.tile([MB, NCOL], f32, tag="s01")
                s23 = opool.tile([MB, NCOL], f32, tag="s23")
                nc.vector.tensor_add(s01[:], lns[0][:], lns[1][:])
                nc.vector.tensor_add(s23[:], lns[2][:], lns[3][:])
                t = opool.tile([MB, NCOL], f32, tag="t")
                nc.vector.tensor_add(t[:], s01[:], s23[:])
                ob = opool.tile([MB, NCOL], f32, tag="ob")
                nc.vector.tensor_scalar(
                    ob[:], t[:], -B_CONST, A_EFF, mult, add
                )
                nc.sync.dma_start(out[row0:row0 + MB, :], ob[:])

    nc.compile()
    return nc


def _prep_weights(kernel):
    """Host prep: fp8 DoubleRow Toeplitz weights + Ln biases."""
    k = np.asarray(kernel)[0].astype(np.float64)        # (4, 16, 16)
    alpha = k[3] / PIXEL_MAX
    w = (1.0 - alpha) ** 2                              # (16, 16)
    d = 2.0 * w[None] * (128.0 - k)                     # (4, 16, 16)

    wq2 = np.asarray(2.0 * w, F8).astype(np.float64)    # quantized 2w taps
    dq8 = np.asarray(d / 8.0, F8).astype(np.float64)    # quantized d/8 taps

    # W8[k, c, dj, t, m]: t=0 band of 2w[:, dj], t=1 band of d_c[:, dj]/8
    w8 = np.zeros((KP, C, Hk, 2, MBP), np.float64)
    idx = np.arange(MB)
    for dj in range(Hk):
        for di in range(Hk):
            w8[idx + di, :, dj, 0, idx] = wq2[di, dj]
            for c in range(C):
                w8[idx + di, c, dj, 1, idx] = dq8[c, di, dj]
    w8 = np.ascontiguousarray(w8.reshape(KP, C * Hk * 2 * MBP)).astype(F8)

    const = (16384.0 * w.sum() - 256.0 * (k * w[None]).sum(axis=(1, 2))
             + (k * k * w[None]).sum(axis=(1, 2)))      # (4,)
    biases = const / 64.0
    return w8, biases


def kernel(x, kernel):
    from concourse.bass_utils import run_bass_kernel_spmd

    x = np.asarray(x)
    w8, biases = _prep_weights(kernel)
    nc = _build_nc(biases)

    x0 = np.ascontiguousarray(x[0], np.float32)         # (4, 2048, 2048)
    in_maps = []
    for r in range(2):
        for cc in range(4):
            r0, c0 = ROW_STARTS[r], COL_STARTS[cc]
            in_maps.append({
                "xs": np.ascontiguousarray(
                    x0[:, r0:r0 + IN_ROWS, c0:c0 + IN_COLS]
                ),
                "w8": w8,
            })

    res = run_bass_kernel_spmd(nc, in_maps, core_ids=list(range(N_CORES)))

    full = np.empty((HO, WO), np.float32)
    for r in range(2):
        for cc in range(4):
            core = r * 4 + cc
            r0, c0 = ROW_STARTS[r], COL_STARTS[cc]
            full[r0:r0 + OUT_ROWS, c0:c0 + NCOL] = res.results[core]["out"]
    return full
